# revision 1
# baseline (speedup 1.0000x reference)
"""Trainium2 Bass kernel: MergedQKVParallelLinearWithLoRA.

out = x @ w_qkv.T + concat_s( lora_expand_s( lora_shrink_s(x)[token's lora] ) )

Strategy (8 NeuronCores, tensor-parallel on the merged QKV output dim):
  - Each core owns 768 of the 6144 output columns: base weight shard
    w_qkv[o0:o1], plus the matching zero-padded LoRA-B shard.
  - x is replicated; tokens are pre-sorted by LoRA id on the host so every
    128-token tile touches 1 (rarely 2) LoRA groups. The permutation is
    applied host-side when laying out x^T, and inverted host-side on the
    output, so the device kernel sees plain contiguous tiles.
  - All matmuls run in float32r (TF32-class: full PE rate at free-dim>=256,
    ~2e-4 rel err) with fp32 PSUM accumulation.
  - Per 256-token supertile: shrinkT[l] = A_cat[l] @ x_tile^T ([48, 256]),
    then per 128-token tile the base matmul (K=4096) and the LoRA expand
    (K=48, zero-padded B ties each of the 3 qkv slices to its columns)
    accumulate into one PSUM tile, DMA'd straight to DRAM.

The kernel is specialized at build time to the token->lora grouping
(group boundaries are baked into the instruction stream); `kernel()`
re-derives them from token_lora_idx on every call, so it is correct for
arbitrary inputs of the fixed shapes below.
"""

import numpy as np

import concourse.mybir as mybir
import concourse.tile as tile
from concourse import bacc, bass_utils

# Walrus ships with LDWEIGHTS dedup disabled; consecutive matmuls on the
# same stationary tile then reload it each time. Enabling it halves LDW
# traffic (verified: identical numerics, 5486->3368 LDWEIGHTS).
if not getattr(bass_utils, "_ldw_opt_patched", False):
    _orig_run_command = bass_utils.run_command

    def _run_command_ldw_opt(argv, **kw):
        argv = ["--enable-ldw-opt=true" if a == "--enable-ldw-opt=false" else a
                for a in argv]
        return _orig_run_command(argv, **kw)

    bass_utils.run_command = _run_command_ldw_opt
    bass_utils._ldw_opt_patched = True

T, D = 8192, 4096
L, R = 8, 16
OUT_SLICES = (4096, 1024, 1024)
O = sum(OUT_SLICES)          # 6144
NCORES = 8
OS = O // NCORES             # 768 output cols per core
P = 128
KT = D // P                  # 32 k-tiles
ST = 256                     # tokens per supertile (>=256 keeps fp32r full-rate)
NST = T // ST
RC = 3 * R                   # 48 stacked lora-rank rows (q,k,v)
N0 = 512                     # base matmul free-dim split: 512 + 256

F32 = mybir.dt.float32
F32R = mybir.dt.float32r

LAST_RESULT = None           # BassKernelResults of the most recent run


def _schedule(sorted_idx: np.ndarray):
    """Per-supertile list of (lora, a, b) token sub-ranges (a/b rel. to supertile)."""
    sched = []
    for st in range(NST):
        win = sorted_idx[st * ST : (st + 1) * ST]
        segs = []
        a = 0
        for i in range(1, ST + 1):
            if i == ST or win[i] != win[a]:
                segs.append((int(win[a]), a, i))
                a = i
        sched.append(segs)
    return sched


def _build(sched):
    max_segs = max(len(s) for s in sched)
    lora_bufs = max(3, max_segs + 1)

    nc = bacc.Bacc("TRN2", target_bir_lowering=False, debug=False,
                   num_devices=NCORES)
    d_x = nc.dram_tensor("xT", [NST, P, KT, ST], F32R, kind="ExternalInput")
    d_w = nc.dram_tensor("wT", [P, KT, OS], F32R, kind="ExternalInput")
    d_a = nc.dram_tensor("aT", [L, P, KT, RC], F32R, kind="ExternalInput")
    d_b = nc.dram_tensor("B", [L, RC, OS], F32R, kind="ExternalInput")
    d_o = nc.dram_tensor("out", [T, OS], F32, kind="ExternalOutput")

    with tile.TileContext(nc) as tc:
        with (
            tc.tile_pool(name="wpool", bufs=1) as wpool,
            tc.tile_pool(name="xpool", bufs=2) as xpool,
            tc.tile_pool(name="apool", bufs=lora_bufs) as apool,
            tc.tile_pool(name="bpool", bufs=lora_bufs) as bpool,
            tc.tile_pool(name="shrpool", bufs=lora_bufs) as shrpool,
            tc.tile_pool(name="opool", bufs=3) as opool,
            tc.tile_pool(name="bpsum", bufs=3, space="PSUM") as bpsum,
            tc.tile_pool(name="spsum", bufs=2, space="PSUM") as spsum,
        ):
            wtr = wpool.tile([P, KT, OS], F32R)
            zt = wpool.tile([RC, ST], F32)   # zeros for boundary-seg padding
            nc.vector.memset(zt[:], 0.0)

            cur = {}  # lora -> (at_r, bt_r) live SBUF tiles
            for st, segs in enumerate(sched):
                xtr = xpool.tile([P, KT, ST], F32R, tag="xt")
                # First supertile: fine-grained x chunks + the first lora's
                # A/B up front, with the 12.6MB weight load interleaved so
                # everything spreads across DMA queues and arrives
                # k-progressively.
                XCH = 4 if st == 0 else 8
                if st == 0:
                    l0 = segs[0][0]
                    at0 = apool.tile([P, KT, RC], F32R, tag="at")
                    bt0 = bpool.tile([RC, OS], F32R, tag="bt")
                    cur[l0] = (at0, bt0)
                for ci, k0 in enumerate(range(0, KT, XCH)):
                    nc.sync.dma_start(
                        xtr[:, k0 : k0 + XCH, :],
                        d_x[st, :, k0 : k0 + XCH, :],
                    )
                    if st == 0:
                        # k-progressive arrival of everything the first
                        # supertile needs: x, A (for shrink), w (for base);
                        # B is only needed by the expand ~30us in.
                        nc.sync.dma_start(at0[:, k0 : k0 + XCH, :],
                                          d_a[l0, :, k0 : k0 + XCH, :])
                        nc.sync.dma_start(wtr[:, ci * 4 : ci * 4 + 4, :],
                                          d_w[:, ci * 4 : ci * 4 + 4, :])
                        if ci == 3:
                            nc.sync.dma_start(bt0[:], d_b[l0])

                seginfo = []
                new = {}
                for (l, a, b) in segs:
                    if l in cur:
                        at_r, bt_r = cur[l]
                    else:
                        at_r = apool.tile([P, KT, RC], F32R, tag="at")
                        nc.sync.dma_start(at_r[:], d_a[l])
                        bt_r = bpool.tile([RC, OS], F32R, tag="bt")
                        nc.sync.dma_start(bt_r[:], d_b[l])
                    new[l] = (at_r, bt_r)

                    # shrinkT[l] = A_cat[l] @ x^T  -> [RC, tokens]
                    ps = spsum.tile([RC, ST], F32, tag="ps")
                    if 4 * (b - a) >= ST:
                        sl = slice(0, ST)
                    else:
                        # fp32r matmuls need aligned/even APs; round to x8.
                        sl = slice((a // 8) * 8, min(ST, -(-b // 8) * 8))
                    for k in range(KT):
                        nc.tensor.matmul(
                            ps[:, sl], at_r[:, k, :], xtr[:, k, sl],
                            start=(k == 0), stop=(k == KT - 1),
                        )
                    sb = shrpool.tile([RC, ST], F32R, tag="sb")
                    if len(segs) > 1:
                        if a > 0:
                            nc.vector.tensor_copy(sb[:, :a], zt[:, :a])
                        if b < ST:
                            nc.vector.tensor_copy(sb[:, b:], zt[:, b:])
                        nc.vector.tensor_copy(sb[:, a:b], ps[:, a:b])
                    else:
                        nc.vector.tensor_copy(sb[:], ps[:])
                    seginfo.append((a, b, sb, bt_r))
                cur = new

                # k-loop interleaved across both 128-token tiles: each w
                # k-chunk feeds 2x the PE work before the next is needed,
                # halving the w-arrival pressure in the early window.
                pbs = [bpsum.tile([P, OS], F32, tag="pb", name=f"pb_{st}_{j}")
                       for j in range(ST // P)]
                for k in range(KT):
                    for j in range(ST // P):
                        t0 = j * P
                        nc.tensor.matmul(
                            pbs[j][:, 0:N0], xtr[:, k, t0 : t0 + P],
                            wtr[:, k, 0:N0], start=(k == 0), stop=False,
                        )
                        nc.tensor.matmul(
                            pbs[j][:, N0:OS], xtr[:, k, t0 : t0 + P],
                            wtr[:, k, N0:OS], start=(k == 0), stop=False,
                        )
                for j in range(ST // P):
                    t0 = j * P
                    pb = pbs[j]
                    over = [s for s in seginfo if s[0] < t0 + P and s[1] > t0]
                    for i, (a, b, sb_r, bt_r) in enumerate(over):
                        last = i == len(over) - 1
                        nc.tensor.matmul(
                            pb[:, 0:N0], sb_r[:, t0 : t0 + P], bt_r[:, 0:N0],
                            start=False, stop=last,
                        )
                        nc.tensor.matmul(
                            pb[:, N0:OS], sb_r[:, t0 : t0 + P], bt_r[:, N0:OS],
                            start=False, stop=last,
                        )
                    ob = opool.tile([P, OS], F32, tag="ob")
                    nc.vector.tensor_copy(ob[:], pb[:])
                    row0 = st * ST + t0
                    nc.sync.dma_start(d_o[row0 : row0 + P, :], ob[:])

    nc.compile()
    return nc


def _prep(x, w_qkv, lora_a, lora_b_q, lora_b_k, lora_b_v, perm):
    x = np.ascontiguousarray(x, dtype=np.float32)
    # Supertile-major layout: xT[st, p, kt, t] = x[perm][st*ST+t, kt*128+p],
    # so each supertile's DMA reads 32KB/partition fully contiguous.
    xT = np.ascontiguousarray(
        x[perm].T.reshape(KT, P, NST, ST).transpose(2, 1, 0, 3)
    )
    w_shards = []
    for c in range(NCORES):
        wc = w_qkv[c * OS : (c + 1) * OS].astype(np.float32, copy=False)
        w_shards.append(np.ascontiguousarray(
            wc.T.reshape(KT, P, OS).transpose(1, 0, 2)
        ))
    # aT[l, p, kt, rc] = lora_a[s, l, r, kt*128+p],  rc = 16*s + r
    a_cat = np.ascontiguousarray(lora_a.transpose(1, 0, 2, 3)).reshape(L, RC, D)
    aT = np.ascontiguousarray(
        a_cat.transpose(2, 0, 1).reshape(KT, P, L, RC).transpose(2, 1, 0, 3)
    )
    # Zero-padded B: rows 16s..16s+16 only hit slice-s columns.
    bfull = np.zeros((L, RC, O), np.float32)
    off = 0
    for s, (bs, osz) in enumerate(
        zip((lora_b_q, lora_b_k, lora_b_v), OUT_SLICES)
    ):
        bfull[:, 16 * s : 16 * (s + 1), off : off + osz] = bs.transpose(0, 2, 1)
        off += osz
    b_shards = [np.ascontiguousarray(bfull[:, :, c * OS : (c + 1) * OS])
                for c in range(NCORES)]
    return xT, w_shards, aT, b_shards


def kernel(x, w_qkv, lora_a, lora_b_q, lora_b_k, lora_b_v, token_lora_idx):
    global LAST_RESULT
    idx = np.asarray(token_lora_idx)
    perm = np.argsort(idx, kind="stable")
    sched = _schedule(idx[perm])

    nc = _build(sched)
    xT, w_shards, aT, b_shards = _prep(
        np.asarray(x), np.asarray(w_qkv), np.asarray(lora_a),
        np.asarray(lora_b_q), np.asarray(lora_b_k), np.asarray(lora_b_v), perm,
    )
    in_maps = [
        {"xT": xT, "wT": w_shards[c], "aT": aT, "B": b_shards[c]}
        for c in range(NCORES)
    ]
    res = bass_utils.run_bass_kernel_spmd(
        nc, in_maps, core_ids=list(range(NCORES))
    )
    LAST_RESULT = res
    out_perm = np.concatenate([res.results[c]["out"] for c in range(NCORES)],
                              axis=1)
    out = np.empty((T, O), np.float32)
    out[perm] = out_perm
    return out



# revision 5
# speedup vs baseline: 1.1961x; 1.1961x over previous
"""Trainium2 Bass kernel: MergedQKVParallelLinearWithLoRA.

out = x @ w_qkv.T + concat_s( lora_expand_s( lora_shrink_s(x)[token's lora] ) )

Strategy (8 NeuronCores, TOKEN-parallel):
  - Tokens are sorted by LoRA id on the host; each core owns a contiguous
    1024-token chunk and computes ALL 6144 output columns for it.  This
    removes the fully-replicated LoRA shrink a column-parallel split pays
    (each core shrinks only its own tokens: 13.6us instead of 109us of PE
    time), at the cost of streaming the full base weight per core.
  - All matmuls run in bf16 (fp32 PSUM accumulation, ~2e-4 rel err vs the
    2e-2 gate): same PE rate as fp32r, half the DMA bytes, and FWL halves
    LDWEIGHTS time.
  - A sorted 1024-token chunk crosses at most a few LoRA boundaries.  Each
    core gets 2*G LoRA "slots" (A/B weights + per-token 0/1 masks) as DATA,
    so the instruction stream is identical on every core (SPMD) and
    independent of where the boundaries fall.  Two slots' A matrices stack
    side-by-side in the 128-wide PE array (2x48=96 cols), so a 2-slot
    shrink/expand costs the same PE time as 1 slot.
  - Per 128-col output chunk oc: psum[128, 1024] accumulates the K=4096
    base matmul (w chunk stationary, x moving) then the LoRA expand
    (masked shrink as moving), and is copied out once.

kernel() re-derives the slot count G from token_lora_idx on every call, so
it is correct for arbitrary inputs of the fixed shapes below.
"""

import ml_dtypes
import numpy as np

import concourse.mybir as mybir
import concourse.tile as tile
from concourse import bacc, bass_utils

T, D = 8192, 4096
L, R = 8, 16
OUT_SLICES = (4096, 1024, 1024)
O = sum(OUT_SLICES)          # 6144
NCORES = 8
TC = T // NCORES             # 1024 tokens per core
P = 128
KT = D // P                  # 32 k-tiles
OC = O // P                  # 48 output-column chunks of 128
RC = 3 * R                   # 48 stacked lora-rank rows (q,k,v)
SLOT2 = 2 * RC               # 96: two lora slots side by side in the array
SLOTP = P                    # slot dim padded to 128 (LDW opt wants full-width loads)
HTC = TC // 2                # 512: psum-bank-sized half of the token dim

F32 = mybir.dt.float32
BF16 = mybir.dt.bfloat16
NPBF16 = ml_dtypes.bfloat16

LAST_RESULT = None           # BassKernelResults of the most recent run


def _core_segments(sorted_idx):
    """Per-core list of (lora, a, b) token sub-ranges (a/b rel. to chunk)."""
    out = []
    for c in range(NCORES):
        win = sorted_idx[c * TC : (c + 1) * TC]
        segs = []
        a = 0
        for i in range(1, TC + 1):
            if i == TC or win[i] != win[a]:
                segs.append((int(win[a]), a, i))
                a = i
        out.append(segs)
    return out


def _build(G):
    nc = bacc.Bacc("TRN2", target_bir_lowering=False, debug=False,
                   num_devices=NCORES)
    d_x = nc.dram_tensor("xT", [P, KT, TC], BF16, kind="ExternalInput")
    d_w = nc.dram_tensor("wT", [OC, P, KT, P], BF16, kind="ExternalInput")
    d_a = nc.dram_tensor("aT", [G, P, KT, SLOTP], BF16, kind="ExternalInput")
    d_b = nc.dram_tensor("B", [G, SLOTP, O], BF16, kind="ExternalInput")
    d_m = nc.dram_tensor("M", [G, SLOTP, TC], BF16, kind="ExternalInput")
    d_o = nc.dram_tensor("out", [O, TC], F32, kind="ExternalOutput")

    # PSUM budget (8 banks of 512 f32): base/expand tiles are 2 banks each,
    # shrink tiles 2 banks each.
    n_po = 3 if G == 1 else 2
    n_ps = min(G, 2)

    with tile.TileContext(nc) as tc:
        with (
            tc.tile_pool(name="xpool", bufs=1) as xpool,
            tc.tile_pool(name="cpool", bufs=1) as cpool,
            tc.tile_pool(name="wpool", bufs=4) as wpool,
            tc.tile_pool(name="opool", bufs=3) as opool,
            tc.tile_pool(name="bpsum", bufs=n_po, space="PSUM") as bpsum,
            tc.tile_pool(name="spsum", bufs=n_ps, space="PSUM") as spsum,
        ):
            at = [cpool.tile([P, KT, SLOTP], BF16, name=f"at{g}")
                  for g in range(G)]
            bt = [cpool.tile([SLOTP, O], BF16, name=f"bt{g}")
                  for g in range(G)]
            mt = [cpool.tile([SLOTP, TC], BF16, name=f"mt{g}")
                  for g in range(G)]
            sbs = [cpool.tile([SLOTP, TC], BF16, name=f"sb{g}")
                   for g in range(G)]
            for g in range(G):
                nc.sync.dma_start(at[g][:], d_a[g])
                nc.sync.dma_start(bt[g][:], d_b[g])
                nc.sync.dma_start(mt[g][:], d_m[g])

            # x arrives k-progressively so the shrink k-loop can start
            # as soon as the first slices land.
            xt = xpool.tile([P, KT, TC], BF16)
            XCH = 2
            for k0 in range(0, KT, XCH):
                nc.sync.dma_start(xt[:, k0 : k0 + XCH, :],
                                  d_x[:, k0 : k0 + XCH, :])

            # shrink: [96, 1024] = A_pair^T @ x, then mask-select per token
            for g in range(G):
                ps = spsum.tile([SLOTP, TC], F32, tag="ps")
                for k in range(KT):
                    nc.tensor.matmul(ps[:, 0:HTC], at[g][:, k, :],
                                     xt[:, k, 0:HTC],
                                     start=(k == 0), stop=(k == KT - 1))
                    nc.tensor.matmul(ps[:, HTC:TC], at[g][:, k, :],
                                     xt[:, k, HTC:TC],
                                     start=(k == 0), stop=(k == KT - 1))
                nc.vector.tensor_tensor(sbs[g][:], ps[:], mt[g][:],
                                        mybir.AluOpType.mult)

            # base + expand, one 128-col output chunk at a time
            for oc in range(OC):
                wt = wpool.tile([P, KT, P], BF16, tag="wt")
                nc.sync.dma_start(wt[:], d_w[oc])
                po = bpsum.tile([P, TC], F32, tag="po")
                for k in range(KT):
                    nc.tensor.matmul(po[:, 0:HTC], wt[:, k, :],
                                     xt[:, k, 0:HTC],
                                     start=(k == 0), stop=False)
                    nc.tensor.matmul(po[:, HTC:TC], wt[:, k, :],
                                     xt[:, k, HTC:TC],
                                     start=(k == 0), stop=False)
                for g in range(G):
                    last = g == G - 1
                    nc.tensor.matmul(po[:, 0:HTC],
                                     bt[g][:, oc * P : (oc + 1) * P],
                                     sbs[g][:, 0:HTC],
                                     start=False, stop=last)
                    nc.tensor.matmul(po[:, HTC:TC],
                                     bt[g][:, oc * P : (oc + 1) * P],
                                     sbs[g][:, HTC:TC],
                                     start=False, stop=last)
                ob = opool.tile([P, TC], F32, tag="ob")
                nc.vector.tensor_copy(ob[:], po[:])
                nc.sync.dma_start(d_o[oc * P : (oc + 1) * P, :], ob[:])

    nc.compile()
    return nc


def _prep(x, w_qkv, lora_a, lora_b_q, lora_b_k, lora_b_v, perm, core_segs, G):
    # xT[c][p, k, t] = x[perm[c*TC+t], k*128+p]
    xs = x[perm].astype(NPBF16)
    x_shards = [
        np.ascontiguousarray(
            xs[c * TC : (c + 1) * TC].T.reshape(KT, P, TC).transpose(1, 0, 2)
        )
        for c in range(NCORES)
    ]
    # wT[oc, p, k, c] = w_qkv[oc*128+c, k*128+p]  (same for every core)
    w_re = np.ascontiguousarray(
        w_qkv.astype(NPBF16).T.reshape(KT, P, OC, P).transpose(2, 1, 0, 3)
    )
    # aT_all[l][p, k, rc] = lora_a[s, l, r, k*128+p],  rc = 16*s + r
    a_cat = np.ascontiguousarray(
        lora_a.transpose(1, 0, 2, 3)
    ).reshape(L, RC, D).astype(NPBF16)
    aT_all = np.ascontiguousarray(
        a_cat.transpose(2, 0, 1).reshape(KT, P, L, RC).transpose(2, 1, 0, 3)
    )  # [L, P, KT, RC]
    # Zero-padded B: rows 16s..16s+16 only hit slice-s columns.
    bfull = np.zeros((L, RC, O), NPBF16)
    off = 0
    for s, (bs, osz) in enumerate(
        zip((lora_b_q, lora_b_k, lora_b_v), OUT_SLICES)
    ):
        bfull[:, 16 * s : 16 * (s + 1), off : off + osz] = (
            bs.transpose(0, 2, 1).astype(NPBF16)
        )
        off += osz

    a_sh, b_sh, m_sh = [], [], []
    for c in range(NCORES):
        a_c = np.zeros((G, P, KT, SLOTP), NPBF16)
        b_c = np.zeros((G, SLOTP, O), NPBF16)
        m_c = np.zeros((G, SLOTP, TC), NPBF16)
        for j, (l, a, b) in enumerate(core_segs[c]):
            g, lane = j // 2, j % 2
            a_c[g, :, :, lane * RC : (lane + 1) * RC] = aT_all[l]
            b_c[g, lane * RC : (lane + 1) * RC, :] = bfull[l]
            m_c[g, lane * RC : (lane + 1) * RC, a:b] = 1.0
        a_sh.append(a_c)
        b_sh.append(b_c)
        m_sh.append(m_c)
    return x_shards, w_re, a_sh, b_sh, m_sh


def kernel(x, w_qkv, lora_a, lora_b_q, lora_b_k, lora_b_v, token_lora_idx):
    global LAST_RESULT
    idx = np.asarray(token_lora_idx)
    perm = np.argsort(idx, kind="stable")
    core_segs = _core_segments(idx[perm])
    G = (max(len(s) for s in core_segs) + 1) // 2

    nc = _build(G)
    x_shards, w_re, a_sh, b_sh, m_sh = _prep(
        np.asarray(x, dtype=np.float32), np.asarray(w_qkv, dtype=np.float32),
        np.asarray(lora_a), np.asarray(lora_b_q), np.asarray(lora_b_k),
        np.asarray(lora_b_v), perm, core_segs, G,
    )
    in_maps = [
        {"xT": x_shards[c], "wT": w_re, "aT": a_sh[c], "B": b_sh[c],
         "M": m_sh[c]}
        for c in range(NCORES)
    ]
    res = bass_utils.run_bass_kernel_spmd(
        nc, in_maps, core_ids=list(range(NCORES))
    )
    LAST_RESULT = res
    out_sorted = np.concatenate(
        [res.results[c]["out"] for c in range(NCORES)], axis=1
    )  # [O, T] in sorted-token order
    out = np.empty((T, O), np.float32)
    out[perm] = out_sorted.T
    return out


# revision 9
# speedup vs baseline: 1.2709x; 1.0626x over previous
"""Trainium2 Bass kernel: MergedQKVParallelLinearWithLoRA.

out = x @ w_qkv.T + concat_s( lora_expand_s( lora_shrink_s(x)[token's lora] ) )

Strategy (8 NeuronCores, TOKEN-parallel):
  - Tokens are grouped by LoRA id on the host; each core owns a 1024-token
    chunk and computes ALL 6144 output columns for it.  This removes the
    fully-replicated LoRA shrink a column-parallel split pays (each core
    shrinks only its own tokens), at the cost of streaming the full base
    weight per core (48MB bf16, hidden under 690us of PE work).
  - The LoRA groups are ORDERED (8! search) so every 1024-token chunk
    touches at most 2 LoRAs; each core gets 2*G LoRA "slots" (A/B weights
    + per-token 0/1 masks) as DATA, so the instruction stream is identical
    on every core (SPMD) regardless of where group boundaries fall.  Two
    slots' A matrices stack side-by-side in the 128-wide PE array, so a
    2-slot shrink/expand costs the same PE time as 1 slot.
  - All matmuls run in bf16 (fp32 PSUM accumulation, ~2e-3 rel err vs the
    2e-2 gate): same PE rate as fp32r, half the DMA bytes, FWL-fast
    LDWEIGHTS.
  - Per 128-col output chunk oc: psum[128, 1024] accumulates the K=4096
    base matmul (w chunk stationary, x moving) then the LoRA expand
    (masked shrink as moving), and is copied out once (split across the
    Scalar and Vector engines).
  - x lands as eight 4-k-slice tiles so the shrink k-loop starts as soon
    as the first slice arrives; the first two ocs' k-loops are interleaved
    with the shrink to keep the PE busy during the x/w DMA fill.

kernel() re-derives the slot count G from token_lora_idx on every call, so
it is correct for arbitrary inputs of the fixed shapes below.
"""

import itertools

import ml_dtypes
import numpy as np

import concourse.mybir as mybir
import concourse.tile as tile
from concourse import bacc, bass_utils

T, D = 8192, 4096
L, R = 8, 16
OUT_SLICES = (4096, 1024, 1024)
O = sum(OUT_SLICES)          # 6144
NCORES = 8
TC = T // NCORES             # 1024 tokens per core
P = 128
KT = D // P                  # 32 k-tiles
OC = O // P                  # 48 output-column chunks of 128
RC = 3 * R                   # 48 stacked lora-rank rows (q,k,v)
SLOTP = P                    # 2x48 slot rows padded to 128
HTC = TC // 2                # 512: psum-bank-sized half of the token dim
XCH = 4                      # k-slices per x tile (separate DMA/dep units)
NXT = KT // XCH              # 8 x tiles

F32 = mybir.dt.float32
BF16 = mybir.dt.bfloat16
NPBF16 = ml_dtypes.bfloat16

LAST_RESULT = None           # BassKernelResults of the most recent run


def _order_loras(counts):
    """Order the lora groups so the max #groups overlapping any 1024-token
    chunk is minimized (8! brute force, ~40k orders)."""
    present = [l for l in range(L) if counts[l] > 0]
    best, best_ms = list(range(L)), 10**9
    bounds = [(c * TC, (c + 1) * TC) for c in range(NCORES)]
    for order in itertools.permutations(present):
        p = 0
        maxseg = 0
        # segments per chunk via interval overlap
        segs = [0] * NCORES
        ok = True
        for l in order:
            q = p + counts[l]
            c0, c1 = p // TC, (q - 1) // TC
            for c in range(c0, c1 + 1):
                segs[c] += 1
                if segs[c] > maxseg:
                    maxseg = segs[c]
            p = q
            if maxseg >= best_ms:
                ok = False
                break
        if ok and maxseg < best_ms:
            best_ms, best = maxseg, list(order)
            if best_ms <= 2:
                break
    return best


def _core_segments(ordered_idx):
    """Per-core list of (lora, a, b) token sub-ranges (a/b rel. to chunk)."""
    out = []
    for c in range(NCORES):
        win = ordered_idx[c * TC : (c + 1) * TC]
        segs = []
        a = 0
        for i in range(1, TC + 1):
            if i == TC or win[i] != win[a]:
                segs.append((int(win[a]), a, i))
                a = i
        out.append(segs)
    return out


def _build(G):
    nc = bacc.Bacc("TRN2", target_bir_lowering=False, debug=False,
                   num_devices=NCORES)
    d_x = nc.dram_tensor("xT", [NXT, P, XCH, TC], BF16, kind="ExternalInput")
    d_w = nc.dram_tensor("wT", [OC, P, KT, P], BF16, kind="ExternalInput")
    d_a = nc.dram_tensor("aT", [G, P, KT, SLOTP], BF16, kind="ExternalInput")
    d_b = nc.dram_tensor("B", [G, SLOTP, O], BF16, kind="ExternalInput")
    d_m = nc.dram_tensor("M", [G, SLOTP, TC], BF16, kind="ExternalInput")
    d_o = nc.dram_tensor("out", [O, TC], F32, kind="ExternalOutput")

    # PSUM budget (8 banks of 512 f32): every psum tile here is 2 banks.
    # spsum holds G named shrink tiles (bufs=1); bpsum cycles n_po "po"
    # slots shared by the early ocs and the main loop.
    n_po = 3 if G == 1 else 2
    n_early = 2 if G <= 2 else 0

    with tile.TileContext(nc) as tc:
        with (
            tc.tile_pool(name="xpool", bufs=1) as xpool,
            tc.tile_pool(name="cpool", bufs=1) as cpool,
            tc.tile_pool(name="wpool", bufs=4) as wpool,
            tc.tile_pool(name="opool", bufs=3) as opool,
            tc.tile_pool(name="bpsum", bufs=n_po, space="PSUM") as bpsum,
            tc.tile_pool(name="spsum", bufs=1, space="PSUM") as spsum,
        ):
            at = [cpool.tile([P, KT, SLOTP], BF16, name=f"at{g}")
                  for g in range(G)]
            bt = [cpool.tile([SLOTP, O], BF16, name=f"bt{g}")
                  for g in range(G)]
            mt = [cpool.tile([SLOTP, TC], BF16, name=f"mt{g}")
                  for g in range(G)]
            sbs = [cpool.tile([SLOTP, TC], BF16, name=f"sb{g}")
                   for g in range(G)]
            xts = [xpool.tile([P, XCH, TC], BF16, name=f"x{i}")
                   for i in range(NXT)]
            wts_e = [wpool.tile([P, KT, P], BF16, name=f"wte{i}")
                     for i in range(n_early)]

            # DMA issue order = arrival order: everything the interleaved
            # k-loop needs first, then the rest.
            for g in range(G):
                nc.sync.dma_start(at[g][:], d_a[g])
            nc.sync.dma_start(xts[0][:], d_x[0])
            for i in range(n_early):
                nc.sync.dma_start(wts_e[i][:], d_w[i])
            for i in range(1, NXT):
                nc.sync.dma_start(xts[i][:], d_x[i])
            for g in range(G):
                nc.sync.dma_start(mt[g][:], d_m[g])
            for g in range(G):
                nc.sync.dma_start(bt[g][:], d_b[g])

            def xk(k):
                return xts[k // XCH][:, k % XCH, :]

            def base_k(po, wt, k):
                nc.tensor.matmul(po[:, 0:HTC], wt[:, k, :], xk(k)[:, 0:HTC],
                                 start=(k == 0), stop=False)
                nc.tensor.matmul(po[:, HTC:TC], wt[:, k, :], xk(k)[:, HTC:TC],
                                 start=(k == 0), stop=False)

            def finish_oc(oc, po):
                for g in range(G):
                    last = g == G - 1
                    bsl = bt[g][:, oc * P : (oc + 1) * P]
                    nc.tensor.matmul(po[:, 0:HTC], bsl, sbs[g][:, 0:HTC],
                                     start=False, stop=last)
                    nc.tensor.matmul(po[:, HTC:TC], bsl, sbs[g][:, HTC:TC],
                                     start=False, stop=last)
                ob = opool.tile([P, TC], F32, tag="ob")
                nc.scalar.activation(ob[:, 0:HTC], po[:, 0:HTC],
                                     mybir.ActivationFunctionType.Copy)
                nc.vector.tensor_copy(ob[:, HTC:TC], po[:, HTC:TC])
                nc.sync.dma_start(d_o[oc * P : (oc + 1) * P, :], ob[:])

            if G <= 2:
                # shrink + first ocs, interleaved per k: the PE chews on
                # these while x/w stream in.
                pss = [spsum.tile([SLOTP, TC], F32, name=f"ps{g}")
                       for g in range(G)]
                pos_e = [bpsum.tile([P, TC], F32, tag="po", name=f"poe{i}")
                         for i in range(n_early)]
                for k in range(KT):
                    for g in range(G):
                        nc.tensor.matmul(pss[g][:, 0:HTC], at[g][:, k, :],
                                         xk(k)[:, 0:HTC],
                                         start=(k == 0), stop=(k == KT - 1))
                        nc.tensor.matmul(pss[g][:, HTC:TC], at[g][:, k, :],
                                         xk(k)[:, HTC:TC],
                                         start=(k == 0), stop=(k == KT - 1))
                    for i in range(n_early):
                        base_k(pos_e[i], wts_e[i], k)
                for g in range(G):
                    nc.vector.tensor_tensor(sbs[g][:], pss[g][:], mt[g][:],
                                            mybir.AluOpType.mult)
                for i in range(n_early):
                    finish_oc(i, pos_e[i])
            else:
                # rare fallback (>4 loras in one chunk): sequential shrink
                for g in range(G):
                    ps = spsum.tile([SLOTP, TC], F32, tag="ps")
                    for k in range(KT):
                        nc.tensor.matmul(ps[:, 0:HTC], at[g][:, k, :],
                                         xk(k)[:, 0:HTC],
                                         start=(k == 0), stop=(k == KT - 1))
                        nc.tensor.matmul(ps[:, HTC:TC], at[g][:, k, :],
                                         xk(k)[:, HTC:TC],
                                         start=(k == 0), stop=(k == KT - 1))
                    nc.vector.tensor_tensor(sbs[g][:], ps[:], mt[g][:],
                                            mybir.AluOpType.mult)

            for oc in range(n_early, OC):
                wt = wpool.tile([P, KT, P], BF16, tag="wt")
                nc.sync.dma_start(wt[:], d_w[oc])
                po = bpsum.tile([P, TC], F32, tag="po")
                for k in range(KT):
                    base_k(po, wt, k)
                finish_oc(oc, po)

    nc.compile()
    return nc


def _prep(x, w_qkv, lora_a, lora_b_q, lora_b_k, lora_b_v, perm, core_segs, G):
    # xT[c][i, p, j, t] = x[perm[c*TC+t], (i*XCH+j)*128+p]
    xs = x[perm].astype(NPBF16)
    x_shards = [
        np.ascontiguousarray(
            xs[c * TC : (c + 1) * TC].T.reshape(NXT, XCH, P, TC)
            .transpose(0, 2, 1, 3)
        )
        for c in range(NCORES)
    ]
    # wT[oc, p, k, c] = w_qkv[oc*128+c, k*128+p]  (same for every core)
    w_re = np.ascontiguousarray(
        w_qkv.astype(NPBF16).T.reshape(KT, P, OC, P).transpose(2, 1, 0, 3)
    )
    # aT_all[l][p, k, rc] = lora_a[s, l, r, k*128+p],  rc = 16*s + r
    a_cat = np.ascontiguousarray(
        lora_a.transpose(1, 0, 2, 3)
    ).reshape(L, RC, D).astype(NPBF16)
    aT_all = np.ascontiguousarray(
        a_cat.transpose(2, 0, 1).reshape(KT, P, L, RC).transpose(2, 1, 0, 3)
    )  # [L, P, KT, RC]
    # Zero-padded B: rows 16s..16s+16 only hit slice-s columns.
    bfull = np.zeros((L, RC, O), NPBF16)
    off = 0
    for s, (bs, osz) in enumerate(
        zip((lora_b_q, lora_b_k, lora_b_v), OUT_SLICES)
    ):
        bfull[:, 16 * s : 16 * (s + 1), off : off + osz] = (
            bs.transpose(0, 2, 1).astype(NPBF16)
        )
        off += osz

    a_sh, b_sh, m_sh = [], [], []
    for c in range(NCORES):
        a_c = np.zeros((G, P, KT, SLOTP), NPBF16)
        b_c = np.zeros((G, SLOTP, O), NPBF16)
        m_c = np.zeros((G, SLOTP, TC), NPBF16)
        for j, (l, a, b) in enumerate(core_segs[c]):
            g, lane = j // 2, j % 2
            a_c[g, :, :, lane * RC : (lane + 1) * RC] = aT_all[l]
            b_c[g, lane * RC : (lane + 1) * RC, :] = bfull[l]
            m_c[g, lane * RC : (lane + 1) * RC, a:b] = 1.0
        a_sh.append(a_c)
        b_sh.append(b_c)
        m_sh.append(m_c)
    return x_shards, w_re, a_sh, b_sh, m_sh


def kernel(x, w_qkv, lora_a, lora_b_q, lora_b_k, lora_b_v, token_lora_idx):
    global LAST_RESULT
    idx = np.asarray(token_lora_idx)
    counts = np.bincount(idx, minlength=L)
    order = _order_loras(counts)
    perm = np.concatenate(
        [np.flatnonzero(idx == l) for l in order if counts[l] > 0]
    )
    core_segs = _core_segments(idx[perm])
    G = (max(len(s) for s in core_segs) + 1) // 2

    nc = _build(G)
    x_shards, w_re, a_sh, b_sh, m_sh = _prep(
        np.asarray(x, dtype=np.float32), np.asarray(w_qkv, dtype=np.float32),
        np.asarray(lora_a), np.asarray(lora_b_q), np.asarray(lora_b_k),
        np.asarray(lora_b_v), perm, core_segs, G,
    )
    in_maps = [
        {"xT": x_shards[c], "wT": w_re, "aT": a_sh[c], "B": b_sh[c],
         "M": m_sh[c]}
        for c in range(NCORES)
    ]
    res = bass_utils.run_bass_kernel_spmd(
        nc, in_maps, core_ids=list(range(NCORES))
    )
    LAST_RESULT = res
    out_sorted = np.concatenate(
        [res.results[c]["out"] for c in range(NCORES)], axis=1
    )  # [O, T] in grouped-token order
    out = np.empty((T, O), np.float32)
    out[perm] = out_sorted.T
    return out
